# revision 7
# baseline (speedup 1.0000x reference)
"""EdgeConv (ParticleNet-style) Trainium2 kernel.

Full inputs: x [128, 512, 32] fp32, W1 [64, 128], b1 [128], W2 [128, 64], b2 [64].
Output: [128, 512, 64] fp32.

Data-parallel over batch: 16 events per core on 8 cores, software-pipelined
(stage1(e) interleaved with stage2(e-2)). Per event:
  stage1:
  - keys[i, j] = ci.cj - |cj|^2/2 via bf16 hi/lo-split matmul (contraction 8,
    host-prepped keyl/keyr), diag self-mask via bf16 ident @ diag accumulate.
    keys columns are host-permuted to v_dram physical row order so selection
    indices are directly gather row ids.
  - top-16 per row: DVE max8 -> match_replace -> max8; indices via max_index.
  - idx regroup to the dma_gather layout [128, 512] (wrap-16, replicated x8)
    entirely on PE+ACT: uint16 -> fp32 cast, 8 one-hot fold matmuls into
    PSUM, one strided ACT cast back to uint16 (no per-element DMA storms).
  - v' = x @ W1b + b1 node-major bf16, written contiguously to DRAM (ACT-issued).
  stage2:
  - gpsimd dma_gather of 8 x 1024 edges (256B rows) on 4 SWDGE queues.
  - h = relu(vg + p) in place (DVE broadcast add, relu split ACT/DVE).
  - hbarT[h, i] = sum_k relu_k^T via PE transpose-accumulate;
    out.T[64, 512] = (W2/16)^T @ hbarT + b2 rank-1; host transposes back.

Key perf facts: HWDGE dma_start descriptor-gen costs ~1.5us on the issuing
sequencer (keep the Sync queue to ~2 DMAs/event); SWDGE gather gen is
~2.3ns/idx when rings do not back up (4 queues).
"""

import numpy as np
import ml_dtypes

B, N, F = 128, 512, 32
K = 16
H, OUT = 128, 64
NCORES = 8
EV = B // NCORES
DIAG_NEG = -1.0e4       # diag mask; far below any real key, fp16-safe
MR_NEG = -3.0e4         # match_replace fill; below DIAG_NEG

SEL_FP16 = False         # selection in fp16 (2x DVE) vs fp32
RELU_ACT_COLS = 2048    # columns of relu evac done on ACT; rest on DVE

_cache = {}


def _build_nc(n_ev=EV):
    import concourse.bass as bass
    import concourse.bacc as bacc
    import concourse.tile as tile
    import concourse.mybir as mybir
    from contextlib import ExitStack

    dt = mybir.dt
    sel_dt = dt.float16 if SEL_FP16 else dt.float32
    nc = bacc.Bacc("TRN2", target_bir_lowering=False, debug=False,
                   enable_asserts=False, num_devices=NCORES,
                   num_swdge_queues=4)

    # DRAM I/O (per core)
    inp_d = nc.dram_tensor("inp", [n_ev, F + 1, N], dt.bfloat16, kind="ExternalInput")
    klr_d = nc.dram_tensor("klr", [n_ev, 8, 2 * N], dt.bfloat16, kind="ExternalInput")
    wpv_d = nc.dram_tensor("wpv", [F + 1, 2 * H], dt.bfloat16, kind="ExternalInput")
    w2b_d = nc.dram_tensor("w2b", [H, OUT], dt.bfloat16, kind="ExternalInput")
    b2s_d = nc.dram_tensor("b2s", [2, OUT], dt.bfloat16, kind="ExternalInput")
    diag_d = nc.dram_tensor("diag", [128, 4, N], dt.bfloat16, kind="ExternalInput")
    ident_d = nc.dram_tensor("ident", [128, 128], dt.bfloat16, kind="ExternalInput")
    mrep_d = nc.dram_tensor("mrep", [128, 8, 128], dt.float32, kind="ExternalInput")
    out_d = nc.dram_tensor("out", [n_ev, OUT, N], dt.float32, kind="ExternalOutput")

    AF = mybir.ActivationFunctionType

    with tile.TileContext(nc) as tc, ExitStack() as ctx:
        cpool = ctx.enter_context(tc.tile_pool(name="consts", bufs=1))
        ident = cpool.tile([128, 128], dt.bfloat16)
        nc.sync.dma_start(ident[:], ident_d[:])
        mrep = cpool.tile([128, 8, 128], dt.float32)
        nc.sync.dma_start(mrep[:], mrep_d[:])
        diag = cpool.tile([128, 4, N], dt.bfloat16)
        nc.sync.dma_start(diag[:], diag_d[:])
        wpv = cpool.tile([F + 1, 2 * H], dt.bfloat16)
        nc.sync.dma_start(wpv[:], wpv_d[:])
        w2b = cpool.tile([H, OUT], dt.bfloat16)
        nc.sync.dma_start(w2b[:], w2b_d[:])
        b2s = cpool.tile([2, OUT], dt.bfloat16)
        nc.sync.dma_start(b2s[:], b2s_d[:])
        ones2 = cpool.tile([2, N], dt.bfloat16)
        nc.vector.memset(ones2[:], 1.0)

        inp_pool = ctx.enter_context(tc.tile_pool(name="inp", bufs=2))
        klr_pool = ctx.enter_context(tc.tile_pool(name="klr", bufs=2))
        keys_pool = ctx.enter_context(tc.tile_pool(name="keys", bufs=2))
        keys2_pool = ctx.enter_context(tc.tile_pool(name="keys2", bufs=2))
        vals_pool = ctx.enter_context(tc.tile_pool(name="vals", bufs=2))
        idxs_pool = ctx.enter_context(tc.tile_pool(name="idxs", bufs=2))
        idxdg_pool = ctx.enter_context(tc.tile_pool(name="idxdg", bufs=4))
        idxf_pool = ctx.enter_context(tc.tile_pool(name="idxf", bufs=2))
        p_pool = ctx.enter_context(tc.tile_pool(name="p", bufs=4))
        v_pool = ctx.enter_context(tc.tile_pool(name="v", bufs=2))
        vg_pool = ctx.enter_context(tc.tile_pool(name="vg", bufs=4))
        outsb_pool = ctx.enter_context(tc.tile_pool(name="outsb", bufs=2))

        vdram_pool = ctx.enter_context(tc.tile_pool(name="vdram", bufs=4, space="DRAM"))
        kps_pool = ctx.enter_context(tc.tile_pool(name="kps", bufs=2, space="PSUM"))
        pps_pool = ctx.enter_context(tc.tile_pool(name="pps", bufs=1, space="PSUM"))
        hps_pool = ctx.enter_context(tc.tile_pool(name="hps", bufs=1, space="PSUM"))
        ops_pool = ctx.enter_context(tc.tile_pool(name="ops", bufs=1, space="PSUM"))
        hbT_pool = ctx.enter_context(tc.tile_pool(name="hbT", bufs=2))

        LAG = 2
        st = {}     # per-event cross-stage tiles: e -> dict

        def stage1(e):
            # ---- load inputs (one DMA: [xq(33) | keyl(8) | keyr(8)])
            inp = inp_pool.tile([F + 1, N], dt.bfloat16)
            nc.sync.dma_start(inp[:], inp_d[e])
            klr = klr_pool.tile([8, 2 * N], dt.bfloat16)
            nc.sync.dma_start(klr[:], klr_d[e])

            # ---- p and v' (both node-major [i%128, i//128, h])
            pps = pps_pool.tile([128, N], dt.float32)
            for c in range(4):
                nc.tensor.matmul(pps[:, 128 * c:128 * (c + 1)],
                                 inp[0:F, 128 * c:128 * (c + 1)],
                                 wpv[0:F, 0:H], start=True, stop=True)
            p_nm = p_pool.tile([128, 4, H], dt.bfloat16)
            nc.scalar.activation(p_nm[:].opt(), pps[:], AF.Copy)

            vps = pps_pool.tile([128, N], dt.float32)
            for c in range(4):
                nc.tensor.matmul(vps[:, 128 * c:128 * (c + 1)],
                                 inp[0:F + 1, 128 * c:128 * (c + 1)],
                                 wpv[0:F + 1, H:2 * H], start=True, stop=True)
            v_sb = v_pool.tile([128, 4, H], dt.bfloat16)
            nc.scalar.activation(v_sb[:].opt(), vps[:], AF.Copy)
            v_dram = vdram_pool.tile([N, H], dt.bfloat16)
            nc.scalar.dma_start(v_dram[:].rearrange("(q c) h -> q c h", c=4, q=128),
                                v_sb[:])

            # ---- keys + selection per 128-row tile
            keys = keys_pool.tile([128, 4, N], sel_dt)
            keys2 = keys2_pool.tile([128, 4, N], sel_dt)
            vals = vals_pool.tile([128, 64], sel_dt)
            idxs = idxs_pool.tile([128, 64], dt.uint16)
            for t in range(4):
                kps = kps_pool.tile([128, N], dt.float32)
                nc.tensor.matmul(kps[:], klr[:, 128 * t:128 * (t + 1)], klr[:, N:2 * N],
                                 start=True, stop=False)
                nc.tensor.matmul(kps[:], ident[:], diag[:, t, :],
                                 start=False, stop=True)
                kt = keys[:, t, :].opt()
                k2t = keys2[:, t, :].opt()
                nc.scalar.activation(kt, kps[:], AF.Copy)
                v0 = vals[:, 16 * t:16 * t + 8]
                v1 = vals[:, 16 * t + 8:16 * t + 16]
                # idxs col layout: 4*r + t (r = k-slot 0..15) for regroup DMA
                idxs4 = idxs[:].rearrange("p (r t) -> p r t", r=16, t=4)
                i0 = idxs4[:, 0:8, t].opt()
                i1 = idxs4[:, 8:16, t].opt()
                nc.vector.max(v0, kt)
                nc.vector.match_replace(k2t, v0, kt, MR_NEG)
                nc.vector.max(v1, k2t)
                nc.vector.max_index(i0, v0, kt)
                nc.vector.max_index(i1, v1, k2t)

            # ---- regroup idx -> dma_gather layout [16, 512] via PE fold
            # logical edge e2 = k*512 + i  ->  idx partition e2%16, col e2//16
            # idxsm[u, 32r + 8t + w] = idxs[16w + u, 4r + t]
            idxf = idxf_pool.tile([128, 64], dt.float32)
            nc.scalar.activation(idxf[:], idxs[:], AF.Copy)
            fps = kps_pool.tile([128, 8, 64], dt.float32)
            for w in range(8):
                nc.tensor.matmul(fps[:, w, :].opt(), mrep[:, w, :].opt(),
                                 idxf[:], start=True, stop=True)
            idx_dg = idxdg_pool.tile([128, N], dt.uint16)
            src_ap = fps[:].rearrange("P w (r t) -> P w r t", r=16, t=4)
            dst_ap = idx_dg[:].rearrange("P (r t w) -> P w r t", r=16, t=4, w=8)
            nc.scalar.activation(dst_ap, src_ap, AF.Copy)
            st[e] = {"idx_dg": idx_dg, "v_dram": v_dram, "p_nm": p_nm}

        def stage2(e):
            idx_dg = st[e]["idx_dg"]
            v_dram = st[e]["v_dram"]
            p_nm = st[e]["p_nm"]
            # ---- gather v' by idx: 8 x 1024 idx on rotating queues
            # node-major gather: vg[p, r, :] = v'[idx[e2]], e2 = r*128 + p
            vg = vg_pool.tile([128, K * N // 128, H], dt.bfloat16)
            for s8 in range(8):
                nc.gpsimd.dma_gather(
                    out_ap=vg[:, s8 * 8:(s8 + 1) * 8, :],
                    in_ap=v_dram[:],
                    idxs_ap=idx_dg[:, s8 * 64:(s8 + 1) * 64].bitcast(dt.int16),
                    num_idxs=1024,
                    num_idxs_reg=1024,
                    elem_size=H,
                    transpose=False,
                    single_packet=True,
                    queue_num=s8 % 4,
                )

            # ---- edges: h = relu(vg + p_i) node-major, in place
            vg4 = vg[:].rearrange("p (k c) h -> p k c h", k=K, c=4)
            p_b = p_nm[:].rearrange("p (k c) h -> p k c h", k=1, c=4).broadcast_to(
                [128, K, 4, H])
            nc.vector.tensor_tensor(vg4, vg4, p_b, op=mybir.AluOpType.add)
            vgr = vg
            vgf = vg[:].opt()   # [128, 8192]
            ac = RELU_ACT_COLS
            if ac > 0:
                nc.scalar.activation(vgf[:, 0:ac], vgf[:, 0:ac], AF.Relu)
            nc.vector.tensor_scalar_max(vgf[:, ac:K * H * 4], vgf[:, ac:K * H * 4],
                                        0.0)

            # ---- hbar.T via PE transpose-accumulate: psum[h, i-chunk c]
            hps = hps_pool.tile([128, 4, 128], dt.float32)
            for c in range(4):
                for k in range(K):
                    nc.tensor.matmul(hps[:, c, :].opt(),
                                     vgr[:, k * 4 + c, :].opt(),
                                     ident[:], start=(k == 0), stop=(k == K - 1))
            hbT = hbT_pool.tile([128, 4 * 128], dt.bfloat16)
            nc.scalar.activation(hbT[:], hps[:].opt(), AF.Copy)

            # ---- layer 2: out.T[64, N] = W2'.T @ hbar.T + b2
            ops = ops_pool.tile([OUT, N], dt.float32)
            nc.tensor.matmul(ops[:], w2b[:], hbT[:], start=True, stop=False)
            nc.tensor.matmul(ops[:], b2s[:], ones2[:],
                             start=False, stop=True)
            osb = outsb_pool.tile([OUT, N], dt.float32)
            nc.scalar.activation(osb[:], ops[:], AF.Copy)
            nc.sync.dma_start(out_d[e], osb[:])
            del st[e]

        for i in range(n_ev + LAG):
            if i < n_ev:
                stage1(i)
            if i >= LAG:
                stage2(i - LAG)

    nc.compile()
    return nc


def _prep_inputs(x, W1, b1, W2, b2):
    bf16 = ml_dtypes.bfloat16
    x = np.asarray(x, dtype=np.float32)
    Wp = (W1[0:F, :] - W1[F:2 * F, :]).astype(np.float32)
    Wv = W1[F:2 * F, :].astype(np.float32)
    wpv = np.zeros((F + 1, 2 * H), dtype=bf16)
    wpv[0:F, 0:H] = Wp.astype(bf16)
    wpv[0:F, H:2 * H] = Wv.astype(bf16)
    wpv[F, H:2 * H] = b1.astype(bf16)

    w2b = (W2.astype(np.float32) / np.float32(K)).astype(bf16)
    b2f = b2.astype(np.float32)
    b2hi = b2f.astype(bf16)
    b2lo = (b2f - b2hi.astype(np.float32)).astype(bf16)
    b2s = np.stack([b2hi, b2lo]).astype(bf16)

    # v_dram rows are (q, c)-major: physical row r = 4q + c holds node 128c + q.
    # keys columns are permuted to physical order so max_index returns rows.
    nperm = 128 * (np.arange(N) % 4) + (np.arange(N) // 4)
    diag = np.zeros((128, 4, N), dtype=bf16)
    for t in range(4):
        diag[np.arange(128), t, 4 * np.arange(128) + t] = bf16(DIAG_NEG)

    ident = np.eye(128, dtype=bf16)
    mrep = np.zeros((128, 8, 128), dtype=np.float32)
    for w in range(8):
        mrep[16 * w + (np.arange(128) % 16), w, np.arange(128)] = 1.0

    xt = np.ascontiguousarray(x.transpose(0, 2, 1))        # [B, F, N] fp32
    inp = np.ones((B, F + 1, N), dtype=bf16)
    inp[:, 0:F, :] = xt.astype(bf16)
    klr = np.empty((B, 8, 2 * N), dtype=bf16)

    # keys hi/lo split: key[i,j] = ci.cj - |cj|^2/2
    c = xt[:, 0:2, :]                                      # [B, 2, N] fp32
    ch = c.astype(bf16)
    cl = (c - ch.astype(np.float32)).astype(bf16)
    n2 = 0.5 * (c[:, 0] ** 2 + c[:, 1] ** 2)               # [B, N] fp32
    nh = n2.astype(bf16)
    nl = (n2 - nh.astype(np.float32)).astype(bf16)
    keyl = klr[:, :, 0:N]
    keyr = np.empty((B, 8, N), dtype=bf16)
    # products: ahx*bhx + ahx*blx + alx*bhx  (same for y), then -nh - nl
    keyl[:, 0] = ch[:, 0]; keyr[:, 0] = ch[:, 0]
    keyl[:, 1] = ch[:, 0]; keyr[:, 1] = cl[:, 0]
    keyl[:, 2] = cl[:, 0]; keyr[:, 2] = ch[:, 0]
    keyl[:, 3] = ch[:, 1]; keyr[:, 3] = ch[:, 1]
    keyl[:, 4] = ch[:, 1]; keyr[:, 4] = cl[:, 1]
    keyl[:, 5] = cl[:, 1]; keyr[:, 5] = ch[:, 1]
    keyl[:, 6] = bf16(1.0); keyr[:, 6] = -nh
    keyl[:, 7] = bf16(1.0); keyr[:, 7] = -nl
    klr[:, :, N:2 * N] = keyr[:, :, nperm]

    return inp, klr, wpv, w2b, b2s, diag, ident, mrep


def kernel(x, W1, b1, W2, b2):
    from concourse.bass_utils import run_bass_kernel_spmd

    key = "nc"
    if key not in _cache:
        _cache[key] = _build_nc()
    nc = _cache[key]

    inp, klr, wpv, w2b, b2s, diag, ident, mrep = _prep_inputs(
        np.asarray(x), np.asarray(W1), np.asarray(b1),
        np.asarray(W2), np.asarray(b2))

    in_maps = []
    for c in range(NCORES):
        sl = slice(c * EV, (c + 1) * EV)
        in_maps.append({
            "inp": inp[sl], "klr": klr[sl],
            "wpv": wpv, "w2b": w2b, "b2s": b2s, "diag": diag, "ident": ident,
            "mrep": mrep,
        })
    res = run_bass_kernel_spmd(nc, in_maps, list(range(NCORES)))
    outs = [res.results[c]["out"] for c in range(NCORES)]  # [EV, OUT, N]
    full = np.concatenate(outs, axis=0)                    # [B, OUT, N]
    return np.ascontiguousarray(full.transpose(0, 2, 1)).astype(np.float32)
